# revision 19
# baseline (speedup 1.0000x reference)
"""BERT-base (12-layer) forward pass on 8 Trainium2 NeuronCores.

Strategy: data-parallel over batch (B=8 -> 1 sequence per core), no
collectives. Host computes the embedding layer (gather + LayerNorm) and
folds every LayerNorm's gain/bias into the downstream weights, so the
device residual stream is the *normalized* activation z in bf16:

    x = g.z + b   is reconstructed on the fly during residual
    accumulation via one fused scalar_tensor_tensor per tile.

On-device compute: bf16 matmuls with f32 PSUM accumulation; LayerNorm
statistics (mean / mean-square over the feature axis) via ones-vector
matmuls; normalization applied in f32 with the row stats broadcast by a
K=1 matmul (rstd) and a GpSimd partition_broadcast (mean*rstd).

Activation layout on device is transposed ([feature partitions, seq
free]) so weight matrices stream from HBM as natural 128x128 lhsT
blocks. The V projection is produced in sequence-major layout augmented
with a column of ones so the attention context matmul yields softmax
denominators for free. The V projection is emitted *inside* the
attention phase so its matmuls fill the PE bubble while the first
head-pairs' softmax exps run on the Scalar engine.
"""
import sys
import os

if "/opt/trn_rl_repo" not in sys.path:
    sys.path.insert(0, "/opt/trn_rl_repo")

import numpy as np
import ml_dtypes

import concourse.bass as bass
from concourse import bacc
import concourse.tile as tile
from concourse import mybir
from concourse.bass_utils import run_bass_kernel_spmd

F32 = mybir.dt.float32
BF16 = mybir.dt.bfloat16
FP16 = mybir.dt.float16
INT32 = mybir.dt.int32
AF = mybir.ActivationFunctionType
ALU = mybir.AluOpType

# Model dims (hardcoded per problem spec)
B, S, H, NH, L, F = 8, 512, 768, 12, 12, 3072
V, TV, PP = 21128, 2, 512
DH = H // NH            # 64
P = 128
HT = H // P             # 6
FT = F // P             # 24
ST = S // P             # 4
EPS = 1e-12
NCORES = 8

NL = int(os.environ.get("BERT_KERNEL_LAYERS", str(L)))
TAPS = os.environ.get("BERT_DEBUG_TAPS", "0") == "1"

_CACHE = {}


def _build():
    nc = bacc.Bacc("TRN2", target_bir_lowering=False, debug=False)

    # ---- DRAM I/O ----
    WQKVO = nc.dram_tensor("wqkvo", [NL, 4, P, HT, H], FP16, kind="ExternalInput")
    WI = nc.dram_tensor("wi", [NL, P, HT, F], FP16, kind="ExternalInput")
    WIO = nc.dram_tensor("wio", [NL, P, FT, H], FP16, kind="ExternalInput")
    PARAMS = nc.dram_tensor("params", [NL, P, 76], F32, kind="ExternalInput")
    BVREP = nc.dram_tensor("bvrep", [NL, P, H], F32, kind="ExternalInput")
    Z0 = nc.dram_tensor("z0", [P, HT, S], FP16, kind="ExternalInput")
    MASKT = nc.dram_tensor("maskt", [P, ST], F32, kind="ExternalInput")
    OUT = nc.dram_tensor("out", [H, S], F32, kind="ExternalOutput")
    if TAPS:
        TQ = nc.dram_tensor("tq", [P, HT, S], FP16, kind="ExternalOutput")
        TK = nc.dram_tensor("tk", [P, HT, S], FP16, kind="ExternalOutput")
        TV = nc.dram_tensor("tv", [P, ST, NH * (DH + 1)], FP16, kind="ExternalOutput")
        TC = nc.dram_tensor("tc", [P, HT, S], FP16, kind="ExternalOutput")
        TP1 = nc.dram_tensor("tp1", [P, HT, S], F32, kind="ExternalOutput")
        TZ1 = nc.dram_tensor("tz1", [P, HT, S], FP16, kind="ExternalOutput")
        TH = nc.dram_tensor("th", [P, FT, S], FP16, kind="ExternalOutput")

    with tile.TileContext(nc) as tc:
        with (
            tc.tile_pool(name="const", bufs=1) as cpool,
            tc.tile_pool(name="act", bufs=1) as apool,       # big per-layer activations
            tc.tile_pool(name="res", bufs=2) as rpool,       # residual z ping-pong
            tc.tile_pool(name="rows", bufs=1) as rows,       # [1,S] stats rows
            tc.tile_pool(name="bcast", bufs=1) as bcp,       # broadcast + LN temps
            tc.tile_pool(name="bv", bufs=1) as bvpool,
            tc.tile_pool(name="ep", bufs=4) as eppool,       # exp tiles (2 head-pairs)
            tc.tile_pool(name="pp", bufs=1, space="PSUM") as pp,
            tc.tile_pool(name="psc", bufs=2, space="PSUM") as psc,
            tc.tile_pool(name="pcx", bufs=2, space="PSUM") as pcx,
        ):
            # ---- persistent constants ----
            params = cpool.tile([P, NL, 76], F32, tag="params")
            nc.sync.dma_start(params[:], PARAMS[:].rearrange("l p c -> p l c"))
            maskt = cpool.tile([P, ST], F32, tag="maskt")
            nc.sync.dma_start(maskt[:], MASKT[:])
            ones_b = cpool.tile([P, 1], FP16, tag="ones")
            nc.any.memset(ones_b[:], 1.0)
            epsc = cpool.tile([P, 1], F32, tag="epsc")
            nc.any.memset(epsc[:], EPS)
            ones_bh = cpool.tile([P, 1], FP16, tag="onesbh")
            nc.any.memset(ones_bh[:], 1.0 / H)
            ones_row = cpool.tile([1, P], FP16, tag="onesrow")
            nc.any.memset(ones_row[:], 1.0)

            # ---- persistent activations ----
            qTb = apool.tile([P, HT, S], FP16, tag="qTb")
            kTb = apool.tile([P, HT, S], FP16, tag="kTb")
            ctxTb = apool.tile([P, HT, S], FP16, tag="ctxTb")
            v_aug = apool.tile([P, ST, NH * (DH + 1)], FP16, tag="v_aug")
            hTb = apool.tile([P, FT, S], FP16, tag="hTb")
            preF = apool.tile([P, HT, S], F32, tag="preF")

            # ones columns of v_aug (written once; per-layer V writes leave them)
            va_view = v_aug[:].rearrange("p st (h d) -> p st h d", d=DH + 1)
            nc.any.memset(va_view[:, :, :, DH : DH + 1], 1.0)

            dummy = cpool.tile([1, 1], F32, tag="dummy")

            def warm(func):
                nc.scalar.activation(dummy[:], epsc[0:1, 0:1], func)

            # residual z for layer 0 comes from the host (embedding LN output)
            z_cur = rpool.tile([P, HT, S], FP16, tag="z")
            nc.sync.dma_start(z_cur[:], Z0[:])

            # ============ helpers ============
            def stats_step(pst, nt):
                """Accumulate LN stats for preF tile nt: pst row 0 = mean,
                row 64 = mean of squares (1/H folded into operands)."""
                inb = bcp.tile([P, S], FP16, tag="inb", bufs=2)
                nc.scalar.copy(inb[:], preF[:, nt])
                nc.tensor.matmul(
                    pst[0:1, :], ones_bh[:], inb[:],
                    start=(nt == 0), stop=(nt == HT - 1),
                )
                sq1 = bcp.tile([P, S], FP16, tag="sq1", bufs=2)
                nc.vector.scalar_tensor_tensor(
                    sq1[:], preF[:, nt], 1.0 / H, preF[:, nt], ALU.mult, ALU.mult
                )
                nc.tensor.matmul(
                    pst[64:65, :], ones_b[:], sq1[:],
                    start=(nt == 0), stop=(nt == HT - 1),
                )

            def ln_tail(pst, z_out):
                """Produce z = (x - mean) * rstd in bf16 from accumulated
                stats. rstd broadcast via K=1 matmul (PSUM); mean*rstd
                broadcast via GpSimd (SBUF) so GpSimd handles half the
                per-tile work without touching PSUM."""
                mrow = rows.tile([1, S], F32, tag="mrow", bufs=2)
                nc.vector.tensor_copy(mrow[:], pst[0:1, :])
                m2 = rows.tile([1, S], F32, tag="rtmp", bufs=2)
                nc.vector.tensor_tensor(m2[:], mrow[:], mrow[:], ALU.mult)
                var = rows.tile([1, S], F32, tag="rtmp", bufs=2)
                nc.vector.tensor_tensor(var[:], pst[64:65, :], m2[:], ALU.subtract)
                rbb = rows.tile([1, S], FP16, tag="rbb", bufs=2)
                nc.scalar.activation(
                    rbb[:], var[:], AF.Abs_reciprocal_sqrt, bias=epsc[0:1, :]
                )
                rm = rows.tile([1, S], F32, tag="rm", bufs=2)
                nc.vector.tensor_tensor(rm[:], mrow[:], rbb[:], ALU.mult)
                # rstd broadcast -> PSUM via K=1 matmul; mean*rstd -> GpSimd
                bc = psc.tile([P, S], F32, tag="lbc", bufs=1)
                nc.tensor.matmul(bc[:], ones_row[:], rbb[:], start=True, stop=True)
                mbb = bcp.tile([P, S], F32, tag="mbb", bufs=2)
                nc.gpsimd.partition_broadcast(mbb[:], rm[:])
                for kt in range(HT):
                    t1 = bcp.tile([P, S], F32, tag="lnt", bufs=2)
                    nc.vector.tensor_tensor(t1[:], preF[:, kt], bc[:], ALU.mult)
                    nc.vector.tensor_tensor(
                        z_out[:, kt], t1[:], mbb[:], ALU.subtract
                    )

            # ============ transformer layers ============
            with (
                tc.tile_pool(name="wq", bufs=2) as wqpool,
                tc.tile_pool(name="wf", bufs=2) as wfpool,
                tc.tile_pool(name="wo", bufs=2) as wopool,
            ):
                for l in range(NL):
                    pb = params[:, l, :]

                    # ---- Q, K projections (transposed out) ----
                    scope_qk = nc.named_scope(f"qk_{l}"); scope_qk.__enter__()
                    warm(AF.Exp)
                    for pi, dst in ((0, qTb), (1, kTb)):
                        w = wqpool.tile([P, HT, H], FP16, tag="wqk")
                        nc.sync.dma_start(w[:], WQKVO[l, pi])
                        for nt in range(HT):
                            ps = pp.tile([P, S], F32, tag="proj", bufs=2)
                            for kt in range(HT):
                                nc.tensor.matmul(
                                    ps[:], w[:, kt, P * nt : P * (nt + 1)],
                                    z_cur[:, kt],
                                    start=(kt == 0), stop=(kt == HT - 1),
                                )
                            nc.scalar.activation(
                                dst[:, nt], ps[:], AF.Identity,
                                bias=pb[:, 6 * pi + nt : 6 * pi + nt + 1],
                            )

                    scope_qk.__exit__(None, None, None)
                    scope_at = nc.named_scope(f"attn_{l}"); scope_at.__enter__()

                    # attention with V projection folded into the pipeline:
                    # scores(0), scores(1), V-proj, then ctx(h) interleaved
                    # with scores(h+2).
                    expts = {}

                    def emit_scores(ht):
                        expt_a = eppool.tile([P, ST, S], FP16, tag="expt")
                        expt_b = eppool.tile([P, ST, S], FP16, tag="expt")
                        expts[ht] = (expt_a, expt_b)
                        for kti in range(ST):
                            pss = []
                            for sub in range(2):
                                base = 64 * sub
                                pssc = psc.tile([P, S], F32, tag="sc", bufs=2)
                                nc.tensor.matmul(
                                    pssc[:],
                                    kTb[base : base + DH, ht, P * kti : P * (kti + 1)],
                                    qTb[base : base + DH, ht, :],
                                    start=True, stop=True,
                                )
                                pss.append(pssc)
                            for sub in range(2):
                                nc.scalar.activation(
                                    expts[ht][sub][:, kti], pss[sub][:], AF.Exp,
                                    scale=0.125, bias=maskt[:, kti : kti + 1],
                                )

                    def emit_ctx(ht):
                        for sub in range(2):
                            h = 2 * ht + sub
                            base = 64 * sub
                            expt = expts[ht][sub]
                            ppc = pcx.tile([P, S], F32, tag="cx")
                            for kti in range(ST):
                                nc.tensor.matmul(
                                    ppc[0 : DH + 1, :],
                                    v_aug[:, kti, (DH + 1) * h : (DH + 1) * (h + 1)],
                                    expt[:, kti],
                                    start=(kti == 0), stop=(kti == ST - 1),
                                )
                            srow = rows.tile([1, S], F32, tag="rtmp", bufs=2)
                            nc.scalar.copy(srow[:], ppc[DH : DH + 1, :])
                            rec = rows.tile([1, S], F32, tag="mrow", bufs=2)
                            nc.vector.reciprocal_approx_fast(rec[:], srow[:])
                            recb = bcp.tile([DH, S], F32, tag="recb", bufs=2)
                            nc.gpsimd.partition_broadcast(recb[:], rec[:])
                            nc.vector.tensor_tensor(
                                ctxTb[base : base + DH, ht, :], ppc[0:DH, :], recb[:],
                                ALU.mult,
                            )

                    emit_scores(0)
                    emit_scores(1)

                    # ---- V projection (normal layout, augmented buffer) ----
                    wv = wqpool.tile([P, HT, H], FP16, tag="wqk")
                    nc.sync.dma_start(wv[:], WQKVO[l, 2])
                    bvr = bvpool.tile([P, H], F32, tag="bvr")
                    nc.sync.dma_start(bvr[:], BVREP[l])
                    for st in range(ST):
                        for half in range(2):
                            ps = pp.tile([P, S], F32, tag="proj", bufs=2)
                            for kt in range(HT):
                                nc.tensor.matmul(
                                    ps[:, :384],
                                    z_cur[:, kt, P * st : P * (st + 1)],
                                    wv[:, kt, 384 * half : 384 * (half + 1)],
                                    start=(kt == 0), stop=(kt == HT - 1),
                                )
                            dst3 = va_view[:, st, 6 * half : 6 * (half + 1), 0:DH]
                            src3 = ps[:, :384].rearrange("p (h d) -> p h d", d=DH)
                            bv3 = bvr[:, 384 * half : 384 * (half + 1)].rearrange(
                                "p (h d) -> p h d", d=DH
                            )
                            nc.vector.tensor_tensor(dst3, src3, bv3, ALU.add)

                    for ht in range(HT):
                        if ht + 2 < HT:
                            emit_scores(ht + 2)
                        emit_ctx(ht)

                    if TAPS and l == 0:
                        nc.sync.dma_start(TQ[:], qTb[:])
                        nc.sync.dma_start(TK[:], kTb[:])
                        nc.sync.dma_start(TV[:], v_aug[:])
                        nc.sync.dma_start(TC[:], ctxTb[:])

                    scope_at.__exit__(None, None, None)
                    scope_ao = nc.named_scope(f"ao_{l}"); scope_ao.__enter__()
                    warm(AF.Abs_reciprocal_sqrt)
                    # ---- attention output + residual(g.z+b) ----
                    wao = wqpool.tile([P, HT, H], FP16, tag="wqk")
                    nc.sync.dma_start(wao[:], WQKVO[l, 3])
                    pst1 = pcx.tile([P, S], F32, tag="cx")
                    for nt in range(HT):
                        ps = pp.tile([P, S], F32, tag="proj", bufs=2)
                        for kt in range(HT):
                            nc.tensor.matmul(
                                ps[:], wao[:, kt, P * nt : P * (nt + 1)],
                                ctxTb[:, kt],
                                start=(kt == 0), stop=(kt == HT - 1),
                            )
                        aob = bcp.tile([P, S], F32, tag="aob", bufs=2)
                        nc.scalar.activation(
                            aob[:], ps[:], AF.Identity, bias=pb[:, 18 + nt : 19 + nt]
                        )
                        nc.vector.scalar_tensor_tensor(
                            preF[:, nt], z_cur[:, nt], pb[:, 12 + nt : 13 + nt],
                            aob[:], ALU.mult, ALU.add,
                        )
                        stats_step(pst1, nt)
                    scope_ao.__exit__(None, None, None)
                    scope_l1 = nc.named_scope(f"ln1_{l}"); scope_l1.__enter__()
                    z1 = rpool.tile([P, HT, S], FP16, tag="z")
                    ln_tail(pst1, z1)
                    scope_l1.__exit__(None, None, None)
                    if TAPS and l == 0:
                        nc.sync.dma_start(TP1[:], preF[:])
                        nc.sync.dma_start(TZ1[:], z1[:])

                    scope_f1 = nc.named_scope(f"ffn1_{l}"); scope_f1.__enter__()
                    # ---- FFN intermediate (gelu) ----
                    for quarter in range(4):
                        wih = wfpool.tile([P, HT, F // 4], FP16, tag="wi")
                        nc.sync.dma_start(
                            wih[:],
                            WI[l][:, :, (F // 4) * quarter : (F // 4) * (quarter + 1)],
                        )
                        for ntl in range(6):
                            nt = 6 * quarter + ntl
                            ps = pp.tile([P, S], F32, tag="proj", bufs=2)
                            for kt in range(HT):
                                nc.tensor.matmul(
                                    ps[:], wih[:, kt, P * ntl : P * (ntl + 1)],
                                    z1[:, kt],
                                    start=(kt == 0), stop=(kt == HT - 1),
                                )
                            nc.scalar.activation(
                                hTb[:, nt], ps[:], AF.Gelu,
                                bias=pb[:, 48 + nt : 49 + nt],
                            )

                    scope_f1.__exit__(None, None, None)
                    if TAPS and l == 0:
                        nc.sync.dma_start(TH[:], hTb[:])
                    scope_f2 = nc.named_scope(f"ffn2_{l}"); scope_f2.__enter__()
                    warm(AF.Abs_reciprocal_sqrt)
                    # ---- FFN output + residual(g.z+b) ----
                    pst2 = pcx.tile([P, S], F32, tag="cx")
                    wioh = []
                    for half in range(2):
                        wt = wopool.tile([P, FT // 2, H], FP16, tag="wio")
                        nc.sync.dma_start(
                            wt[:],
                            WIO[l][:, (FT // 2) * half : (FT // 2) * (half + 1), :],
                        )
                        wioh.append(wt)
                    for nt in range(HT):
                        ps = pp.tile([P, S], F32, tag="proj", bufs=2)
                        for half in range(2):
                            for kk in range(FT // 2):
                                kt = (FT // 2) * half + kk
                                nc.tensor.matmul(
                                    ps[:], wioh[half][:, kk, P * nt : P * (nt + 1)],
                                    hTb[:, kt],
                                    start=(kt == 0), stop=(kt == FT - 1),
                                )
                        aob = bcp.tile([P, S], F32, tag="aob", bufs=2)
                        nc.scalar.activation(
                            aob[:], ps[:], AF.Identity, bias=pb[:, 30 + nt : 31 + nt]
                        )
                        nc.vector.scalar_tensor_tensor(
                            preF[:, nt], z1[:, nt], pb[:, 24 + nt : 25 + nt],
                            aob[:], ALU.mult, ALU.add,
                        )
                        stats_step(pst2, nt)
                    scope_f2.__exit__(None, None, None)
                    scope_l2 = nc.named_scope(f"ln2_{l}"); scope_l2.__enter__()
                    z_cur = rpool.tile([P, HT, S], FP16, tag="z")
                    ln_tail(pst2, z_cur)
                    scope_l2.__exit__(None, None, None)

                # ============ output: x = g.z + b of the last LN2 ============
                pbl = params[:, NL - 1, :]
                for kt in range(HT):
                    nc.vector.tensor_scalar(
                        out=preF[:, kt], in0=z_cur[:, kt],
                        scalar1=pbl[:, 36 + kt : 37 + kt],
                        scalar2=pbl[:, 42 + kt : 43 + kt],
                        op0=ALU.mult, op1=ALU.add,
                    )
                nc.sync.dma_start(
                    OUT[:].rearrange("(ht p) s -> p ht s", p=P), preF[:]
                )

    nc.compile()
    return nc


def _r6(v):
    return np.ascontiguousarray(v.reshape(6, P).T)


def _prep_shared(inputs):
    bf = np.float16
    f32 = np.float32

    emb_g = np.asarray(inputs["emb_g"], f32)
    emb_b = np.asarray(inputs["emb_b"], f32)
    ln1_g = np.asarray(inputs["ln1_g"], f32)
    ln1_b = np.asarray(inputs["ln1_b"], f32)
    ln2_g = np.asarray(inputs["ln2_g"], f32)
    ln2_b = np.asarray(inputs["ln2_b"], f32)

    wqkvo = np.empty((NL, 4, P, HT, H), dtype=bf)
    wi = np.empty((NL, P, HT, F), dtype=bf)
    wio = np.empty((NL, P, FT, H), dtype=bf)
    params = np.zeros((NL, P, 76), dtype=f32)
    bvrep = np.empty((NL, P, H), dtype=f32)

    def pack_w(w):  # [H, N] -> [P, HT, N]
        return np.ascontiguousarray(
            w.reshape(HT, P, -1).transpose(1, 0, 2)
        ).astype(bf)

    for l in range(NL):
        gprev = emb_g if l == 0 else ln2_g[l - 1]
        bprev = emb_b if l == 0 else ln2_b[l - 1]
        Wq = np.asarray(inputs["Wq"][l], f32)
        Wk = np.asarray(inputs["Wk"][l], f32)
        Wv = np.asarray(inputs["Wv"][l], f32)
        Wao = np.asarray(inputs["Wao"][l], f32)
        Wi = np.asarray(inputs["Wi"][l], f32)
        Wio = np.asarray(inputs["Wio"][l], f32)

        wqkvo[l, 0] = pack_w(gprev[:, None] * Wq)
        wqkvo[l, 1] = pack_w(gprev[:, None] * Wk)
        wqkvo[l, 2] = pack_w(gprev[:, None] * Wv)
        wqkvo[l, 3] = pack_w(Wao)
        wi[l] = pack_w(ln1_g[l][:, None] * Wi)
        wio[l] = np.ascontiguousarray(
            Wio.reshape(FT, P, H).transpose(1, 0, 2)
        ).astype(bf)

        bq_eff = np.asarray(inputs["bq"][l], f32) + bprev @ Wq
        bk_eff = np.asarray(inputs["bk"][l], f32) + bprev @ Wk
        bv_eff = np.asarray(inputs["bv"][l], f32) + bprev @ Wv
        bi_eff = np.asarray(inputs["bi"][l], f32) + ln1_b[l] @ Wi

        params[l, :, 0:6] = _r6(bq_eff)
        params[l, :, 6:12] = _r6(bk_eff)
        params[l, :, 12:18] = _r6(gprev)
        params[l, :, 18:24] = _r6(np.asarray(inputs["bao"][l], f32) + bprev)
        params[l, :, 24:30] = _r6(ln1_g[l])
        params[l, :, 30:36] = _r6(np.asarray(inputs["bio"][l], f32) + ln1_b[l])
        params[l, :, 36:42] = _r6(ln2_g[l])
        params[l, :, 42:48] = _r6(ln2_b[l])
        params[l, :, 48:72] = bi_eff.reshape(FT, P).T
        bvrep[l] = np.broadcast_to(bv_eff, (P, H))

    return {
        "wqkvo": wqkvo, "wi": wi, "wio": wio, "params": params, "bvrep": bvrep,
    }


def _prep_cores(inputs):
    """Per-core inputs: host-computed embedding LN (normalized, no g/b)
    and the additive attention-mask rows."""
    f32 = np.float32
    bf = np.float16
    ids = np.asarray(inputs["input_ids"], np.int32)
    seg = np.asarray(inputs["segment_ids"], np.int32)
    mask = np.asarray(inputs["attention_mask"], f32)
    tok = np.asarray(inputs["tok_emb"], f32)
    typ = np.asarray(inputs["type_emb"], f32)
    pos = np.asarray(inputs["pos_emb"], f32)[:S]

    x = tok[ids] + typ[seg] + pos[None, :, :]       # [B, S, H]
    mu = x.mean(axis=-1, keepdims=True)
    var = x.var(axis=-1, keepdims=True)
    z = (x - mu) / np.sqrt(var + EPS)

    cores = []
    for c in range(B):
        z0 = np.ascontiguousarray(
            z[c].T.reshape(HT, P, S).transpose(1, 0, 2)
        ).astype(bf)
        mrow = (1.0 - mask[c, 0, 0]) * -10000.0
        maskt = np.ascontiguousarray(mrow.reshape(ST, P).T)
        cores.append({"z0": z0, "maskt": maskt})
    return cores


def build_in_maps(inputs):
    shared = _prep_shared(inputs)
    cores = _prep_cores(inputs)
    return [dict(shared, **core) for core in cores]


def kernel(**inputs):
    if "nc" not in _CACHE:
        _CACHE["nc"] = _build()
    nc = _CACHE["nc"]
    in_maps = build_in_maps(inputs)
    res = run_bass_kernel_spmd(nc, in_maps, core_ids=list(range(NCORES)))
    out = np.empty((B, S, H), dtype=np.float32)
    for c in range(NCORES):
        out[c] = res.results[c]["out"].T
    return out


# revision 20
# speedup vs baseline: 1.0658x; 1.0658x over previous
"""BERT-base (12-layer) forward pass on 8 Trainium2 NeuronCores.

Strategy: data-parallel over batch (B=8 -> 1 sequence per core), no
collectives. Host computes the embedding layer (gather + LayerNorm) and
folds every LayerNorm's gain/bias into the downstream weights, so the
device residual stream is the *normalized* activation z in bf16:

    x = g.z + b   is reconstructed on the fly during residual
    accumulation via one fused scalar_tensor_tensor per tile.

On-device compute: bf16 matmuls with f32 PSUM accumulation; LayerNorm
statistics (mean / mean-square over the feature axis) via ones-vector
matmuls; normalization applied in f32 with the row stats broadcast by a
K=1 matmul (rstd) and a GpSimd partition_broadcast (mean*rstd).

Activation layout on device is transposed ([feature partitions, seq
free]) so weight matrices stream from HBM as natural 128x128 lhsT
blocks. The V projection is produced in sequence-major layout augmented
with a column of ones so the attention context matmul yields softmax
denominators for free. The V projection is emitted *inside* the
attention phase so its matmuls fill the PE bubble while the first
head-pairs' softmax exps run on the Scalar engine.
"""
import sys
import os

if "/opt/trn_rl_repo" not in sys.path:
    sys.path.insert(0, "/opt/trn_rl_repo")

import numpy as np
import ml_dtypes

import concourse.bass as bass
from concourse import bacc
import concourse.tile as tile
from concourse import mybir
from concourse.bass_utils import run_bass_kernel_spmd

F32 = mybir.dt.float32
BF16 = mybir.dt.bfloat16
FP16 = mybir.dt.float16
INT32 = mybir.dt.int32
AF = mybir.ActivationFunctionType
ALU = mybir.AluOpType

# Model dims (hardcoded per problem spec)
B, S, H, NH, L, F = 8, 512, 768, 12, 12, 3072
V, TV, PP = 21128, 2, 512
DH = H // NH            # 64
P = 128
HT = H // P             # 6
FT = F // P             # 24
ST = S // P             # 4
EPS = 1e-12
NCORES = 8

NL = int(os.environ.get("BERT_KERNEL_LAYERS", str(L)))
TAPS = os.environ.get("BERT_DEBUG_TAPS", "0") == "1"

_CACHE = {}


def _build():
    nc = bacc.Bacc("TRN2", target_bir_lowering=False, debug=False)

    # ---- DRAM I/O ----
    WQKVO = nc.dram_tensor("wqkvo", [NL, 4, P, HT, H], FP16, kind="ExternalInput")
    WI = nc.dram_tensor("wi", [NL, P, HT, F], FP16, kind="ExternalInput")
    WIO = nc.dram_tensor("wio", [NL, P, FT, H], FP16, kind="ExternalInput")
    PARAMS = nc.dram_tensor("params", [NL, P, 76], F32, kind="ExternalInput")
    BVREP = nc.dram_tensor("bvrep", [NL, P, H], F32, kind="ExternalInput")
    Z0 = nc.dram_tensor("z0", [P, HT, S], FP16, kind="ExternalInput")
    MASKT = nc.dram_tensor("maskt", [P, ST], F32, kind="ExternalInput")
    OUT = nc.dram_tensor("out", [H, S], F32, kind="ExternalOutput")
    if TAPS:
        TQ = nc.dram_tensor("tq", [P, HT, S], FP16, kind="ExternalOutput")
        TK = nc.dram_tensor("tk", [P, HT, S], FP16, kind="ExternalOutput")
        TV = nc.dram_tensor("tv", [P, ST, NH * (DH + 1)], FP16, kind="ExternalOutput")
        TC = nc.dram_tensor("tc", [P, HT, S], FP16, kind="ExternalOutput")
        TP1 = nc.dram_tensor("tp1", [P, HT, S], F32, kind="ExternalOutput")
        TZ1 = nc.dram_tensor("tz1", [P, HT, S], FP16, kind="ExternalOutput")
        TH = nc.dram_tensor("th", [P, FT, S], FP16, kind="ExternalOutput")

    with tile.TileContext(nc) as tc:
        with (
            tc.tile_pool(name="const", bufs=1) as cpool,
            tc.tile_pool(name="act", bufs=1) as apool,       # big per-layer activations
            tc.tile_pool(name="res", bufs=2) as rpool,       # residual z ping-pong
            tc.tile_pool(name="rows", bufs=1) as rows,       # [1,S] stats rows
            tc.tile_pool(name="bcast", bufs=1) as bcp,       # broadcast + LN temps
            tc.tile_pool(name="bv", bufs=1) as bvpool,
            tc.tile_pool(name="ep", bufs=4) as eppool,       # exp tiles (2 head-pairs)
            tc.tile_pool(name="pp", bufs=1, space="PSUM") as pp,
            tc.tile_pool(name="psc", bufs=2, space="PSUM") as psc,
            tc.tile_pool(name="pcx", bufs=2, space="PSUM") as pcx,
        ):
            # ---- persistent constants ----
            params = cpool.tile([P, NL, 76], F32, tag="params")
            nc.sync.dma_start(params[:], PARAMS[:].rearrange("l p c -> p l c"))
            maskt = cpool.tile([P, ST], F32, tag="maskt")
            nc.sync.dma_start(maskt[:], MASKT[:])
            ones_b = cpool.tile([P, 1], FP16, tag="ones")
            nc.any.memset(ones_b[:], 1.0)
            epsc = cpool.tile([P, 1], F32, tag="epsc")
            nc.any.memset(epsc[:], EPS)
            ones_bh = cpool.tile([P, 1], FP16, tag="onesbh")
            nc.any.memset(ones_bh[:], 1.0 / H)
            ones_row = cpool.tile([1, P], FP16, tag="onesrow")
            nc.any.memset(ones_row[:], 1.0)

            # ---- persistent activations ----
            qTb = apool.tile([P, HT, S], FP16, tag="qTb")
            kTb = apool.tile([P, HT, S], FP16, tag="kTb")
            ctxTb = apool.tile([P, HT, S], FP16, tag="ctxTb")
            v_aug = apool.tile([P, ST, NH * (DH + 1)], FP16, tag="v_aug")
            hTb = apool.tile([P, FT, S], FP16, tag="hTb")
            preF = apool.tile([P, HT, S], F32, tag="preF")

            # ones columns of v_aug (written once; per-layer V writes leave them)
            va_view = v_aug[:].rearrange("p st (h d) -> p st h d", d=DH + 1)
            nc.any.memset(va_view[:, :, :, DH : DH + 1], 1.0)

            dummy = cpool.tile([1, 1], F32, tag="dummy")

            def warm(func):
                nc.scalar.activation(dummy[:], epsc[0:1, 0:1], func)

            # residual z for layer 0 comes from the host (embedding LN output)
            z_cur = rpool.tile([P, HT, S], FP16, tag="z")
            nc.sync.dma_start(z_cur[:], Z0[:])

            # ============ helpers ============
            def stats_step(pst, nt):
                """Accumulate LN stats for preF tile nt: pst row 0 = mean,
                row 64 = mean of squares (1/H folded into operands)."""
                inb = bcp.tile([P, S], FP16, tag="inb", bufs=2)
                nc.scalar.copy(inb[:], preF[:, nt])
                nc.tensor.matmul(
                    pst[0:1, :], ones_bh[:], inb[:],
                    start=(nt == 0), stop=(nt == HT - 1),
                )
                sq1 = bcp.tile([P, S], FP16, tag="sq1", bufs=2)
                nc.vector.scalar_tensor_tensor(
                    sq1[:], preF[:, nt], 1.0 / H, preF[:, nt], ALU.mult, ALU.mult
                )
                nc.tensor.matmul(
                    pst[64:65, :], ones_b[:], sq1[:],
                    start=(nt == 0), stop=(nt == HT - 1),
                )

            def ln_tail(pst, z_out):
                """Produce z = (x - mean) * rstd in bf16 from accumulated
                stats. rstd broadcast via K=1 matmul (PSUM); mean*rstd
                broadcast via GpSimd (SBUF) so GpSimd handles half the
                per-tile work without touching PSUM."""
                mrow = rows.tile([1, S], F32, tag="mrow", bufs=2)
                nc.vector.tensor_copy(mrow[:], pst[0:1, :])
                m2 = rows.tile([1, S], F32, tag="rtmp", bufs=2)
                nc.vector.tensor_tensor(m2[:], mrow[:], mrow[:], ALU.mult)
                var = rows.tile([1, S], F32, tag="rtmp", bufs=2)
                nc.vector.tensor_tensor(var[:], pst[64:65, :], m2[:], ALU.subtract)
                rbb = rows.tile([1, S], FP16, tag="rbb", bufs=2)
                nc.scalar.activation(
                    rbb[:], var[:], AF.Abs_reciprocal_sqrt, bias=epsc[0:1, :]
                )
                rm = rows.tile([1, S], F32, tag="rm", bufs=2)
                nc.vector.tensor_tensor(rm[:], mrow[:], rbb[:], ALU.mult)
                # rstd broadcast -> PSUM via K=1 matmul; mean*rstd -> GpSimd
                bc = psc.tile([P, S], F32, tag="lbc", bufs=1)
                nc.tensor.matmul(bc[:], ones_row[:], rbb[:], start=True, stop=True)
                mbb = bcp.tile([P, S], F32, tag="mbb", bufs=2)
                nc.gpsimd.partition_broadcast(mbb[:], rm[:])
                for kt in range(HT):
                    t1 = bcp.tile([P, S], F32, tag="lnt", bufs=2)
                    nc.vector.tensor_tensor(t1[:], preF[:, kt], bc[:], ALU.mult)
                    nc.vector.tensor_tensor(
                        z_out[:, kt], t1[:], mbb[:], ALU.subtract
                    )

            # ============ transformer layers ============
            with (
                tc.tile_pool(name="wq", bufs=2) as wqpool,
                tc.tile_pool(name="wf", bufs=2) as wfpool,
                tc.tile_pool(name="wo", bufs=2) as wopool,
            ):
                for l in range(NL):
                    pb = params[:, l, :]

                    # ---- Q, K projections (transposed out) ----
                    scope_qk = nc.named_scope(f"qk_{l}"); scope_qk.__enter__()
                    warm(AF.Exp)
                    for pi, dst in ((0, qTb), (1, kTb)):
                        w = wqpool.tile([P, HT, H], FP16, tag="wqk")
                        nc.sync.dma_start(w[:], WQKVO[l, pi])
                        for nt in range(HT):
                            ps = pp.tile([P, S], F32, tag="proj", bufs=2)
                            for kt in range(HT):
                                nc.tensor.matmul(
                                    ps[:], w[:, kt, P * nt : P * (nt + 1)],
                                    z_cur[:, kt],
                                    start=(kt == 0), stop=(kt == HT - 1),
                                )
                            nc.scalar.activation(
                                dst[:, nt], ps[:], AF.Identity,
                                bias=pb[:, 6 * pi + nt : 6 * pi + nt + 1],
                            )

                    scope_qk.__exit__(None, None, None)
                    scope_at = nc.named_scope(f"attn_{l}"); scope_at.__enter__()

                    # attention with V projection folded into the pipeline:
                    # scores(0), scores(1), V-proj, then ctx(h) interleaved
                    # with scores(h+2).
                    expts = {}

                    def emit_scores(ht):
                        expt_a = eppool.tile([P, ST, S], FP16, tag="expt")
                        expt_b = eppool.tile([P, ST, S], FP16, tag="expt")
                        expts[ht] = (expt_a, expt_b)
                        for kti in range(ST):
                            pss = []
                            for sub in range(2):
                                base = 64 * sub
                                pssc = psc.tile([P, S], F32, tag="sc", bufs=3)
                                nc.tensor.matmul(
                                    pssc[:],
                                    kTb[base : base + DH, ht, P * kti : P * (kti + 1)],
                                    qTb[base : base + DH, ht, :],
                                    start=True, stop=True,
                                )
                                pss.append(pssc)
                            for sub in range(2):
                                nc.scalar.activation(
                                    expts[ht][sub][:, kti], pss[sub][:], AF.Exp,
                                    scale=0.125, bias=maskt[:, kti : kti + 1],
                                )

                    def emit_ctx(ht):
                        for sub in range(2):
                            h = 2 * ht + sub
                            base = 64 * sub
                            expt = expts[ht][sub]
                            ppc = pcx.tile([P, S], F32, tag="cx")
                            for kti in range(ST):
                                nc.tensor.matmul(
                                    ppc[0 : DH + 1, :],
                                    v_aug[:, kti, (DH + 1) * h : (DH + 1) * (h + 1)],
                                    expt[:, kti],
                                    start=(kti == 0), stop=(kti == ST - 1),
                                )
                            srow = rows.tile([1, S], F32, tag="rtmp", bufs=2)
                            nc.vector.tensor_copy(srow[:], ppc[DH : DH + 1, :])
                            rec = rows.tile([1, S], F32, tag="mrow", bufs=2)
                            nc.vector.reciprocal_approx_fast(rec[:], srow[:])
                            recb = bcp.tile([DH, S], F32, tag="recb", bufs=2)
                            nc.gpsimd.partition_broadcast(recb[:], rec[:])
                            nc.vector.tensor_tensor(
                                ctxTb[base : base + DH, ht, :], ppc[0:DH, :], recb[:],
                                ALU.mult,
                            )

                    emit_scores(0)
                    emit_scores(1)

                    # ---- V projection (normal layout, augmented buffer) ----
                    wv = wqpool.tile([P, HT, H], FP16, tag="wqk")
                    nc.sync.dma_start(wv[:], WQKVO[l, 2])
                    bvr = bvpool.tile([P, H], F32, tag="bvr")
                    nc.sync.dma_start(bvr[:], BVREP[l])
                    for st in range(ST):
                        for half in range(2):
                            ps = pp.tile([P, S], F32, tag="proj", bufs=2)
                            for kt in range(HT):
                                nc.tensor.matmul(
                                    ps[:, :384],
                                    z_cur[:, kt, P * st : P * (st + 1)],
                                    wv[:, kt, 384 * half : 384 * (half + 1)],
                                    start=(kt == 0), stop=(kt == HT - 1),
                                )
                            dst3 = va_view[:, st, 6 * half : 6 * (half + 1), 0:DH]
                            src3 = ps[:, :384].rearrange("p (h d) -> p h d", d=DH)
                            bv3 = bvr[:, 384 * half : 384 * (half + 1)].rearrange(
                                "p (h d) -> p h d", d=DH
                            )
                            nc.vector.tensor_tensor(dst3, src3, bv3, ALU.add)

                    for ht in range(HT):
                        emit_ctx(ht)
                        if ht + 2 < HT:
                            emit_scores(ht + 2)

                    if TAPS and l == 0:
                        nc.sync.dma_start(TQ[:], qTb[:])
                        nc.sync.dma_start(TK[:], kTb[:])
                        nc.sync.dma_start(TV[:], v_aug[:])
                        nc.sync.dma_start(TC[:], ctxTb[:])

                    scope_at.__exit__(None, None, None)
                    scope_ao = nc.named_scope(f"ao_{l}"); scope_ao.__enter__()
                    warm(AF.Abs_reciprocal_sqrt)
                    # ---- attention output + residual(g.z+b) ----
                    wao = wqpool.tile([P, HT, H], FP16, tag="wqk")
                    nc.sync.dma_start(wao[:], WQKVO[l, 3])
                    pst1 = pcx.tile([P, S], F32, tag="cx")
                    for nt in range(HT):
                        ps = pp.tile([P, S], F32, tag="proj", bufs=2)
                        for kt in range(HT):
                            nc.tensor.matmul(
                                ps[:], wao[:, kt, P * nt : P * (nt + 1)],
                                ctxTb[:, kt],
                                start=(kt == 0), stop=(kt == HT - 1),
                            )
                        aob = bcp.tile([P, S], F32, tag="aob", bufs=2)
                        nc.vector.tensor_scalar_add(
                            aob[:], ps[:], pb[:, 18 + nt : 19 + nt]
                        )
                        nc.vector.scalar_tensor_tensor(
                            preF[:, nt], z_cur[:, nt], pb[:, 12 + nt : 13 + nt],
                            aob[:], ALU.mult, ALU.add,
                        )
                        stats_step(pst1, nt)
                    scope_ao.__exit__(None, None, None)
                    scope_l1 = nc.named_scope(f"ln1_{l}"); scope_l1.__enter__()
                    z1 = rpool.tile([P, HT, S], FP16, tag="z")
                    ln_tail(pst1, z1)
                    scope_l1.__exit__(None, None, None)
                    if TAPS and l == 0:
                        nc.sync.dma_start(TP1[:], preF[:])
                        nc.sync.dma_start(TZ1[:], z1[:])

                    scope_f1 = nc.named_scope(f"ffn1_{l}"); scope_f1.__enter__()
                    # ---- FFN intermediate (gelu) ----
                    for quarter in range(4):
                        wih = wfpool.tile([P, HT, F // 4], FP16, tag="wi")
                        nc.sync.dma_start(
                            wih[:],
                            WI[l][:, :, (F // 4) * quarter : (F // 4) * (quarter + 1)],
                        )
                        for ntl in range(6):
                            nt = 6 * quarter + ntl
                            ps = pp.tile([P, S], F32, tag="proj", bufs=2)
                            for kt in range(HT):
                                nc.tensor.matmul(
                                    ps[:], wih[:, kt, P * ntl : P * (ntl + 1)],
                                    z1[:, kt],
                                    start=(kt == 0), stop=(kt == HT - 1),
                                )
                            nc.scalar.activation(
                                hTb[:, nt], ps[:], AF.Gelu,
                                bias=pb[:, 48 + nt : 49 + nt],
                            )

                    scope_f1.__exit__(None, None, None)
                    if TAPS and l == 0:
                        nc.sync.dma_start(TH[:], hTb[:])
                    scope_f2 = nc.named_scope(f"ffn2_{l}"); scope_f2.__enter__()
                    warm(AF.Abs_reciprocal_sqrt)
                    # ---- FFN output + residual(g.z+b) ----
                    pst2 = pcx.tile([P, S], F32, tag="cx")
                    wioh = []
                    for half in range(2):
                        wt = wopool.tile([P, FT // 2, H], FP16, tag="wio")
                        nc.sync.dma_start(
                            wt[:],
                            WIO[l][:, (FT // 2) * half : (FT // 2) * (half + 1), :],
                        )
                        wioh.append(wt)
                    for nt in range(HT):
                        ps = pp.tile([P, S], F32, tag="proj", bufs=2)
                        for half in range(2):
                            for kk in range(FT // 2):
                                kt = (FT // 2) * half + kk
                                nc.tensor.matmul(
                                    ps[:], wioh[half][:, kk, P * nt : P * (nt + 1)],
                                    hTb[:, kt],
                                    start=(kt == 0), stop=(kt == FT - 1),
                                )
                        aob = bcp.tile([P, S], F32, tag="aob", bufs=2)
                        nc.vector.tensor_scalar_add(
                            aob[:], ps[:], pb[:, 30 + nt : 31 + nt]
                        )
                        nc.vector.scalar_tensor_tensor(
                            preF[:, nt], z1[:, nt], pb[:, 24 + nt : 25 + nt],
                            aob[:], ALU.mult, ALU.add,
                        )
                        stats_step(pst2, nt)
                    scope_f2.__exit__(None, None, None)
                    scope_l2 = nc.named_scope(f"ln2_{l}"); scope_l2.__enter__()
                    z_cur = rpool.tile([P, HT, S], FP16, tag="z")
                    ln_tail(pst2, z_cur)
                    scope_l2.__exit__(None, None, None)

                # ============ output: x = g.z + b of the last LN2 ============
                pbl = params[:, NL - 1, :]
                for kt in range(HT):
                    nc.vector.tensor_scalar(
                        out=preF[:, kt], in0=z_cur[:, kt],
                        scalar1=pbl[:, 36 + kt : 37 + kt],
                        scalar2=pbl[:, 42 + kt : 43 + kt],
                        op0=ALU.mult, op1=ALU.add,
                    )
                nc.sync.dma_start(
                    OUT[:].rearrange("(ht p) s -> p ht s", p=P), preF[:]
                )

    nc.compile()
    return nc


def _r6(v):
    return np.ascontiguousarray(v.reshape(6, P).T)


def _prep_shared(inputs):
    bf = np.float16
    f32 = np.float32

    emb_g = np.asarray(inputs["emb_g"], f32)
    emb_b = np.asarray(inputs["emb_b"], f32)
    ln1_g = np.asarray(inputs["ln1_g"], f32)
    ln1_b = np.asarray(inputs["ln1_b"], f32)
    ln2_g = np.asarray(inputs["ln2_g"], f32)
    ln2_b = np.asarray(inputs["ln2_b"], f32)

    wqkvo = np.empty((NL, 4, P, HT, H), dtype=bf)
    wi = np.empty((NL, P, HT, F), dtype=bf)
    wio = np.empty((NL, P, FT, H), dtype=bf)
    params = np.zeros((NL, P, 76), dtype=f32)
    bvrep = np.empty((NL, P, H), dtype=f32)

    def pack_w(w):  # [H, N] -> [P, HT, N]
        return np.ascontiguousarray(
            w.reshape(HT, P, -1).transpose(1, 0, 2)
        ).astype(bf)

    for l in range(NL):
        gprev = emb_g if l == 0 else ln2_g[l - 1]
        bprev = emb_b if l == 0 else ln2_b[l - 1]
        Wq = np.asarray(inputs["Wq"][l], f32)
        Wk = np.asarray(inputs["Wk"][l], f32)
        Wv = np.asarray(inputs["Wv"][l], f32)
        Wao = np.asarray(inputs["Wao"][l], f32)
        Wi = np.asarray(inputs["Wi"][l], f32)
        Wio = np.asarray(inputs["Wio"][l], f32)

        wqkvo[l, 0] = pack_w(gprev[:, None] * Wq)
        wqkvo[l, 1] = pack_w(gprev[:, None] * Wk)
        wqkvo[l, 2] = pack_w(gprev[:, None] * Wv)
        wqkvo[l, 3] = pack_w(Wao)
        wi[l] = pack_w(ln1_g[l][:, None] * Wi)
        wio[l] = np.ascontiguousarray(
            Wio.reshape(FT, P, H).transpose(1, 0, 2)
        ).astype(bf)

        bq_eff = np.asarray(inputs["bq"][l], f32) + bprev @ Wq
        bk_eff = np.asarray(inputs["bk"][l], f32) + bprev @ Wk
        bv_eff = np.asarray(inputs["bv"][l], f32) + bprev @ Wv
        bi_eff = np.asarray(inputs["bi"][l], f32) + ln1_b[l] @ Wi

        params[l, :, 0:6] = _r6(bq_eff)
        params[l, :, 6:12] = _r6(bk_eff)
        params[l, :, 12:18] = _r6(gprev)
        params[l, :, 18:24] = _r6(np.asarray(inputs["bao"][l], f32) + bprev)
        params[l, :, 24:30] = _r6(ln1_g[l])
        params[l, :, 30:36] = _r6(np.asarray(inputs["bio"][l], f32) + ln1_b[l])
        params[l, :, 36:42] = _r6(ln2_g[l])
        params[l, :, 42:48] = _r6(ln2_b[l])
        params[l, :, 48:72] = bi_eff.reshape(FT, P).T
        bvrep[l] = np.broadcast_to(bv_eff, (P, H))

    return {
        "wqkvo": wqkvo, "wi": wi, "wio": wio, "params": params, "bvrep": bvrep,
    }


def _prep_cores(inputs):
    """Per-core inputs: host-computed embedding LN (normalized, no g/b)
    and the additive attention-mask rows."""
    f32 = np.float32
    bf = np.float16
    ids = np.asarray(inputs["input_ids"], np.int32)
    seg = np.asarray(inputs["segment_ids"], np.int32)
    mask = np.asarray(inputs["attention_mask"], f32)
    tok = np.asarray(inputs["tok_emb"], f32)
    typ = np.asarray(inputs["type_emb"], f32)
    pos = np.asarray(inputs["pos_emb"], f32)[:S]

    x = tok[ids] + typ[seg] + pos[None, :, :]       # [B, S, H]
    mu = x.mean(axis=-1, keepdims=True)
    var = x.var(axis=-1, keepdims=True)
    z = (x - mu) / np.sqrt(var + EPS)

    cores = []
    for c in range(B):
        z0 = np.ascontiguousarray(
            z[c].T.reshape(HT, P, S).transpose(1, 0, 2)
        ).astype(bf)
        mrow = (1.0 - mask[c, 0, 0]) * -10000.0
        maskt = np.ascontiguousarray(mrow.reshape(ST, P).T)
        cores.append({"z0": z0, "maskt": maskt})
    return cores


def build_in_maps(inputs):
    shared = _prep_shared(inputs)
    cores = _prep_cores(inputs)
    return [dict(shared, **core) for core in cores]


def kernel(**inputs):
    if "nc" not in _CACHE:
        _CACHE["nc"] = _build()
    nc = _CACHE["nc"]
    in_maps = build_in_maps(inputs)
    res = run_bass_kernel_spmd(nc, in_maps, core_ids=list(range(NCORES)))
    out = np.empty((B, S, H), dtype=np.float32)
    for c in range(NCORES):
        out[c] = res.results[c]["out"].T
    return out
